# revision 50
# baseline (speedup 1.0000x reference)
"""BiCGSTAB (4 fixed iterations, 7-point stencil) on 8 Trainium2 NeuronCores.

Problem: x,b,ref: [2,256,256,256] f32, center: [1,256,256,1] f32.
reference() runs 4 BiCGSTAB iterations of A.u where A is the 7-point stencil
  S(u)[b,h,w,z] = center[h,w]*u - u[w-1] - u[w+1] - u[h-1] - u[h+1] - u[z-1] - u[z+1]
with zero Dirichlet boundaries, and global (per-batch) dot products.

Sharding: core c in 0..7 handles batch b=c//4 and H-slab [64*(c%4), 64*(c%4)+64).
Dot products become 4-rank AllReduces in groups [[0..3],[4..7]]. H-halo planes
are exchanged via AllGather within the group + indirect-DMA ghost-row gathers
(edge cores index a zeroed row range, implementing the Dirichlet boundary).

v3 design (vs v2 baseline, ~528 MB -> ~390 MB HBM traffic per core):
- DRAM fields live in [W, HC, Z] layout so every block/window DMA moves
  4KB-contiguous per-partition lines (v2's [HC, W, Z] layout gave 512B rows).
  Host transposes the x/b slabs in and the x slab out.
- v = lam*S(p) is SBUF-resident for the whole iteration ([128, HC+2, Z] f16
  per W-chunk, ghost planes at rows 0 and HC+1 filled once per iteration by
  indirect-DMA gathers from the AllGather buffer). No v DRAM field at all.
- s and t are never materialized in DRAM: every pass that needs them
  recomputes s = r - alpha*v from r-windows + resident v (two cheap DVE ops)
  and t = lam*S(s) on the otherwise-idle TensorEngine (P45 re-runs the
  stencil instead of loading t back).
- r0 is streamed from DRAM for the <r0,v> / <r0,t> dots (the SBUF freed by
  not keeping r0 resident is what pays for the resident v).
- it0 aliasing: P0 stores r0 once; p- and r-reads of iteration 0 point at it.
- The ENTIRE 7-point stencil runs on the TensorEngine: W-shifts via tridiag
  matmul, H-shifts via +-Z offsets and Z-shifts via +-1 offsets on the
  flattened (h,z) moving operand, accumulated in PSUM. Two tiny DVE ops fix
  the z-wraparound columns. Fields are f16 scaled by lam=1/256 where noted so
  products stay in range; dot results are rescaled after the AllReduce.
"""
import numpy as np

import concourse.bacc as bacc
import concourse.bass as bass
import concourse.bass_isa as bass_isa
import concourse.mybir as mybir
import concourse.tile as tile

F32 = mybir.dt.float32
F16 = mybir.dt.float16
BF16 = mybir.dt.bfloat16
I32 = mybir.dt.int32

N_CORES = 8
GROUP = 4  # cores per batch group
EPS = 1e-6

KH = 8    # h planes per stencil block (fp16 passes)
KH0 = 4   # h planes per block in P0
KH4 = 4   # h planes per update block in P45
# Stencil outputs are computed as lam*S(u) (lam folded into cen and the shift
# matrices, both exact in fp16) so v and t stay inside fp16 range; the scalar
# coefficients compensate (alpha*ILAM etc).
LAM = 1.0 / 256.0
ILAM = 256.0


def build_program(HC=64, W=256, Z=256, ITERS=4, collectives=True,
                  maxph=99, twin_reps=0, fold_center=True, shifts_dve=1):
    """Build the per-core SPMD Bass program. HC = H planes per core.

    collectives=False builds a single-core timing twin (collective_compute
    calls skipped; numerics wrong) usable for wall-clock delta timing.
    """
    assert W == 256 and Z == 256 and HC % KH == 0 and HC % KH0 == 0
    NB = HC // KH
    NB0 = HC // KH0
    RG = [list(range(GROUP)), list(range(GROUP, 2 * GROUP))]
    # halo buffer row layouts (rows = field*2W + side*W + w_global)
    ZR_A = GROUP * 4 * W   # zero-row base in haloA_out (2 fields)
    ZR_B = GROUP * 2 * W   # zero-row base in haloB/C_out (1 field)

    twin = twin_reps > 0
    assert not (twin and collectives), "twin loop cannot contain collectives"
    nc = bacc.Bacc("TRN2", target_bir_lowering=False, debug=False,
                   num_devices=N_CORES)

    if twin:
        x_in = nc.dram_tensor("xin_t", [W, HC, Z], F16)
        b_in = nc.dram_tensor("bin_t", [W, HC, Z], F16)
        x_out = nc.dram_tensor("xout_t", [W, HC, Z], F32)
        dummy_out = nc.dram_tensor("dummy_o", [1, 8], F32, kind="ExternalOutput")
    else:
        # x/b are converted to f16 on the host: halves their DMA traffic and
        # removes the f32->f16 window copies in P0 (error ~5e-4 relative,
        # well inside the f16 noise floor of the field storage)
        x_in = nc.dram_tensor("x", [W, HC, Z], F16, kind="ExternalInput")
        b_in = nc.dram_tensor("bb", [W, HC, Z], F16, kind="ExternalInput")
        x_out = nc.dram_tensor("xout", [W, HC, Z], F32, kind="ExternalOutput")
    cen_in = nc.dram_tensor("cen", [W, HC], F32, kind="ExternalInput")
    matsb_in = nc.dram_tensor("matsb", [128, 640], F16, kind="ExternalInput")
    idxA_in = nc.dram_tensor("idxA", [W, 4], I32, kind="ExternalInput")
    idxB_in = nc.dram_tensor("idxB", [W, 2], I32, kind="ExternalInput")

    with tile.TileContext(nc) as tc:
        with (
            tc.tile_pool(name="sb", bufs=2) as sb,
            tc.tile_pool(name="ps", bufs=8, space="PSUM") as ps,
            tc.tile_pool(name="dr", bufs=1, space="DRAM") as dr,
        ):
            _cnt = [0]

            def _nm(pfx):
                _cnt[0] += 1
                return f"{pfx}{_cnt[0]}"

            # ---- persistent DRAM intermediates ([W, HC, Z] layout)
            fld = {n: dr.tile([W, HC, Z], F16, tag=n, name=f"fld_{n}")
                   for n in ("r0", "p", "t")}
            xw = dr.tile([W, HC, Z], F16, tag="xw", name="fld_xw")
            haloA_in = dr.tile([4 * W, Z], F16, tag="hAi")
            haloA_out = dr.tile([ZR_A + 128, Z], F16, tag="hAo")
            haloB_in = dr.tile([2 * W, Z], F16, tag="hBi")
            haloB_out = dr.tile([ZR_B + 128, Z], F16, tag="hBo")
            din = dr.tile([1, 8], F32, tag="din")
            dout = dr.tile([1, 8], F32, tag="dout")

            # ---- persistent SBUF constants
            cen_sb = []
            for wc in range(2):
                c = sb.tile([128, HC], F32, tag=f"cen{wc}", bufs=1)
                nc.sync.dma_start(out=c[:], in_=cen_in[wc * 128:(wc + 1) * 128, :])
                cen_sb.append(c)
            matsb_sb = sb.tile([128, 640], F16, tag="matsb", bufs=1)
            nc.sync.dma_start(out=matsb_sb[:], in_=matsb_in[:, :])
            idxA_sb = []
            idxB_sb = []
            for wc in range(2):
                ia = sb.tile([128, 4], I32, tag=f"idxA{wc}", bufs=1)
                nc.sync.dma_start(out=ia[:], in_=idxA_in[wc * 128:(wc + 1) * 128, :])
                idxA_sb.append(ia)
                ib = sb.tile([128, 2], I32, tag=f"idxB{wc}", bufs=1)
                nc.sync.dma_start(out=ib[:], in_=idxB_in[wc * 128:(wc + 1) * 128, :])
                idxB_sb.append(ib)
            # resident v = lam*S(p) and resident r: [128, HC+2, Z] per
            # W-chunk, ghost planes at rows 0 and HC+1 (row i holds h-plane
            # i-1). r residency removes all r DRAM traffic (r-windows for
            # the t-stencil, r loads and r_new stores in the update pass).
            v_sb = [sb.tile([128, HC + 2, Z], F16, tag=f"v_{wc}", bufs=1,
                            name=f"vsb{wc}") for wc in range(2)]
            r_sb = [sb.tile([128, HC + 2, Z], F16, tag=f"r_{wc}", bufs=1,
                            name=f"rsb{wc}") for wc in range(2)]

            # zero tails + din
            ztb = sb.tile([128, Z], F16, tag="ghb", name="ztb")
            nc.vector.memset(ztb[:], 0.0)
            nc.sync.dma_start(out=haloA_out[ZR_A:ZR_A + 128, :], in_=ztb[:])
            nc.sync.dma_start(out=haloB_out[ZR_B:ZR_B + 128, :], in_=ztb[:])
            z8 = sb.tile([1, 8], F32, tag="z8", bufs=1)
            nc.vector.memset(z8[:], 0.0)
            nc.sync.dma_start(out=din[:, :], in_=z8[:])

            # matrix APs
            # fold_center layout: [M0 | M1 | B01n | B10n | In]
            # generic layout:     [A  | B01 | B10  | I    | --]
            A_b, B01_b, B10_b, I_b = (matsb_sb[:, 0:128], matsb_sb[:, 128:256],
                                      matsb_sb[:, 256:384], matsb_sb[:, 384:512])
            In_b = matsb_sb[:, 512:640]

            # ---- helpers ------------------------------------------------
            def border_order(nb):
                e = [j for j in range(nb) if 0 < j < nb - 1]
                return e + ([0] if nb == 1 else [0, nb - 1])

            def load_window(field, wc, j, tag, kh, halo_out_t, idx_t, cols,
                            dt_, bufs=None, eng=None):
                """[128, kh+2, Z] window of planes j*kh-1 .. j*kh+kh."""
                h0 = j * kh
                nb = HC // kh
                w0 = wc * 128
                win = sb.tile([128, kh + 2, Z], dt_, tag=tag, name=_nm("win"),
                              bufs=bufs)
                lo_g = (j == 0)
                hi_g = (j == nb - 1)
                a = 0 if lo_g else h0 - 1
                bnd = HC if hi_g else h0 + kh + 1
                po = 1 if lo_g else 0
                (eng or nc.sync).dma_start(
                    out=win[:, po:po + (bnd - a), :],
                    in_=field[w0:w0 + 128, a:bnd, :])
                if lo_g:
                    nc.gpsimd.indirect_dma_start(
                        out=win[:, 0, :], out_offset=None, in_=halo_out_t[:, :],
                        in_offset=bass.IndirectOffsetOnAxis(
                            ap=idx_t[wc][:, cols[0]:cols[0] + 1], axis=0))
                if hi_g:
                    nc.gpsimd.indirect_dma_start(
                        out=win[:, kh + 1, :], out_offset=None,
                        in_=halo_out_t[:, :],
                        in_offset=bass.IndirectOffsetOnAxis(
                            ap=idx_t[wc][:, cols[1]:cols[1] + 1], axis=0))
                return win

            def stencil_tile(wins, wc, j, kh, act_wins=None, au_dt=F16,
                             out_t=None, oh0=0):
                """lam*S(u) for chunk wc, block j, from (win0, win1).

                Result goes to out_t rows [oh0, oh0+kh) if given (e.g. the
                resident v tile), else into a fresh o{wc} tile. Returns
                (tile, row_offset).
                """
                h0 = j * kh
                KZ = kh * Z
                win = wins[wc]
                awin = (act_wins or wins)[wc]
                other = wins[1 - wc]
                wf_m = win[:].rearrange("p h z -> p (h z)")
                of_m = other[:].rearrange("p h z -> p (h z)")
                # h+-1 / z+-1 shifts are free-dim offsets. shifts_dve picks
                # how many pair-sums run as DVE adds (the rest stream as
                # identity matmuls through the PE) to balance engine load:
                # 0: 4 identity MMs; 1: h-pair on DVE (3 MMs); 2: both pairs
                # on DVE (2 MMs); 3: single summed operand (1 MM).
                sd = shifts_dve
                if sd >= 1:
                    sh = sb.tile([128, kh, Z], F16, tag="sh", name=_nm("sh"))
                    shf = sh[:].rearrange("p h z -> p (h z)")
                    nc.vector.tensor_tensor(
                        out=shf[:, 0:KZ], in0=wf_m[:, 0:KZ],
                        in1=wf_m[:, 2 * Z:2 * Z + KZ], op=mybir.AluOpType.add)
                if sd >= 2:
                    sh2 = sb.tile([128, kh, Z], F16, tag="sh2", name=_nm("s2"))
                    sh2f = sh2[:].rearrange("p h z -> p (h z)")
                    nc.vector.tensor_tensor(
                        out=sh2f[:, 0:KZ], in0=wf_m[:, Z - 1:Z - 1 + KZ],
                        in1=wf_m[:, Z + 1:Z + 1 + KZ], op=mybir.AluOpType.add)
                if sd >= 3:
                    nc.vector.tensor_tensor(
                        out=shf[:, 0:KZ], in0=shf[:, 0:KZ],
                        in1=sh2f[:, 0:KZ], op=mybir.AluOpType.add)
                if fold_center:
                    # PSUM accumulates lam*S(u) directly: M holds
                    # diag(lam*c) - lam*tridiag, In/Bn hold -lam shifts.
                    M_ = A_b if wc == 0 else B01_b
                    I_ = In_b
                    Bm = B10_b if wc == 0 else matsb_sb[:, 384:512]
                else:
                    M_, I_ = A_b, I_b
                    Bm = B01_b if wc == 0 else B10_b
                    au = sb.tile([128, kh, Z], au_dt, tag="au", name=_nm("au"))
                    for j1 in range(kh):
                        h = h0 + j1
                        nc.scalar.mul(out=au[:, j1, :],
                                      in_=awin[:, j1 + 1, :],
                                      mul=cen_sb[wc][:, h:h + 1])
                    auf = au[:].rearrange("p h z -> p (h z)")
                if out_t is None:
                    ot = sb.tile([128, kh, Z], F16, tag="oo", name=_nm("vt"))
                    o0_ = 0
                else:
                    ot, o0_ = out_t, oh0
                vf = ot[:, o0_:o0_ + kh, :].rearrange("p h z -> p (h z)")
                # matmul free size is capped at 512 (one PSUM bank) by the
                # s3d3 ISA check, so accumulate per-bank q-tiles
                for q in range(kh * Z // 512):
                    c0, c1 = q * 512, (q + 1) * 512
                    pt = ps.tile([128, 512], F32, tag="pt", name=_nm("pt"))
                    nc.tensor.matmul(out=pt[:], lhsT=M_,
                                     rhs=wf_m[:, Z + c0:Z + c1],
                                     start=True, stop=False)
                    if sd == 0:
                        nc.tensor.matmul(out=pt[:], lhsT=I_,
                                         rhs=wf_m[:, c0:c1],
                                         start=False, stop=False)
                        nc.tensor.matmul(out=pt[:], lhsT=I_,
                                         rhs=wf_m[:, 2 * Z + c0:2 * Z + c1],
                                         start=False, stop=False)
                    else:
                        nc.tensor.matmul(out=pt[:], lhsT=I_,
                                         rhs=shf[:, c0:c1],
                                         start=False, stop=False)
                    if sd <= 1:
                        nc.tensor.matmul(out=pt[:], lhsT=I_,
                                         rhs=wf_m[:, Z + c0 - 1:Z + c1 - 1],
                                         start=False, stop=False)
                        nc.tensor.matmul(out=pt[:], lhsT=I_,
                                         rhs=wf_m[:, Z + c0 + 1:Z + c1 + 1],
                                         start=False, stop=False)
                    elif sd == 2:
                        nc.tensor.matmul(out=pt[:], lhsT=I_,
                                         rhs=sh2f[:, c0:c1],
                                         start=False, stop=False)
                    nc.tensor.matmul(out=pt[:], lhsT=Bm,
                                     rhs=of_m[:, Z + c0:Z + c1],
                                     start=False, stop=True)
                    if fold_center:
                        nc.scalar.copy(out=vf[:, c0:c1], in_=pt[:])
                    else:
                        nc.vector.tensor_tensor(out=vf[:, c0:c1],
                                                in0=auf[:, c0:c1], in1=pt[:],
                                                op=mybir.AluOpType.subtract)
                # undo z-shift wraparound at z=0 / z=Z-1 (scaled by lam)
                nc.vector.scalar_tensor_tensor(
                    out=ot[:, o0_:o0_ + kh, 0:1], in0=win[:, 0:kh, Z - 1:Z],
                    scalar=LAM, in1=ot[:, o0_:o0_ + kh, 0:1],
                    op0=mybir.AluOpType.mult, op1=mybir.AluOpType.add)
                nc.vector.scalar_tensor_tensor(
                    out=ot[:, o0_:o0_ + kh, Z - 1:Z], in0=win[:, 2:kh + 2, 0:1],
                    scalar=LAM, in1=ot[:, o0_:o0_ + kh, Z - 1:Z],
                    op0=mybir.AluOpType.mult, op1=mybir.AluOpType.add)
                return ot, o0_

            def ttr(in0, in1, acc_prev, tag="accA"):
                # dot-product partial: scr = LAM*in0*in1 (discarded), acc row
                # sums. LAM keeps the fp16 products in range; the reduced
                # dots are rescaled after the AllReduce. DVE only:
                # TensorScalarPtr is not a valid Pool-engine opcode (walrus
                # rejects it at codegen even though bass/CoreSim accept it).
                scr = sb.tile([128, KH, Z], F16, tag="au", name=_nm("scr"),
                              bufs=1)
                sf = scr[:].rearrange("p h z -> p (h z)")
                n = in0.free_size()
                acc = sb.tile([128, 1], F32, tag=tag + "p", bufs=4,
                              name=_nm("acc"))
                nc.vector.scalar_tensor_tensor(
                    out=sf[:, 0:n], in0=in0, scalar=LAM, in1=in1,
                    op0=mybir.AluOpType.mult, op1=mybir.AluOpType.mult,
                    accum_out=acc[:])
                if acc_prev is None:
                    return acc
                tot = sb.tile([128, 1], F32, tag=tag, bufs=4, name=_nm("accs"))
                nc.vector.tensor_add(out=tot[:], in0=acc_prev[:], in1=acc[:])
                return tot

            def finish_dot(acc, col):
                red = sb.tile([128, 1], F32, tag="dscp", bufs=8,
                              name=_nm("red"))
                nc.gpsimd.partition_all_reduce(red[:], acc[:], channels=128,
                                               reduce_op=bass_isa.ReduceOp.add)
                nc.sync.dma_start(out=din[0:1, col:col + 1], in_=red[0:1, 0:1])

            def allreduce():
                if collectives:
                    nc.gpsimd.collective_compute(
                        "AllReduce", mybir.AluOpType.add, replica_groups=RG,
                        ins=[din[:, :].opt()], outs=[dout[:, :].opt()])
                else:
                    # twin: keep dsb finite (dout is never collective-written)
                    nc.sync.dma_start(out=dout[:, :], in_=din[:, :])
                dsb = sb.tile([1, 8], F32, tag="dsb", bufs=6, name=_nm("dsb"))
                nc.sync.dma_start(out=dsb[:], in_=dout[:, :])
                return dsb

            def allgather(halo_in_t, halo_out_t, zr):
                if not collectives:
                    return
                nc.gpsimd.collective_compute(
                    "AllGather", mybir.AluOpType.bypass, replica_groups=RG,
                    ins=[halo_in_t[:, :].opt()],
                    outs=[halo_out_t[0:zr, :].opt()])

            def stage_plane(src_plane, halo_in_t, f, side, wc):
                r0_ = f * 2 * W + side * W + wc * 128
                nc.sync.dma_start(out=halo_in_t[r0_:r0_ + 128, :],
                                  in_=src_plane)

            def s_tile():
                return sb.tile([1, 1], F32, tag="dsc", bufs=16, name=_nm("sc"))

            def s_recip_eps(a_ap):
                t_ = s_tile()
                nc.vector.tensor_scalar_add(out=t_[:], in0=a_ap, scalar1=EPS)
                r_ = s_tile()
                nc.vector.reciprocal(out=r_[:], in_=t_[:])
                return r_

            def s_mul(a_ap, b_ap):
                t_ = s_tile()
                nc.vector.tensor_tensor(out=t_[:], in0=a_ap, in1=b_ap,
                                        op=mybir.AluOpType.mult)
                return t_

            def s_sub(a_ap, b_ap):
                t_ = s_tile()
                nc.vector.tensor_tensor(out=t_[:], in0=a_ap, in1=b_ap,
                                        op=mybir.AluOpType.subtract)
                return t_

            def s_scale(a_ap, imm):
                t_ = s_tile()
                nc.vector.tensor_scalar_mul(out=t_[:], in0=a_ap, scalar1=imm)
                return t_

            def bcast(a_ap):
                b_ = sb.tile([128, 1], F32, tag="bc", bufs=8, name=_nm("bc"))
                nc.gpsimd.partition_broadcast(b_[:], a_ap, channels=128)
                return b_

            def stt(out, in0, sc, in1, eng=None):
                """out = in0*sc + in1 (sc: [128,1] AP)."""
                (eng or nc.vector).scalar_tensor_tensor(
                    out=out, in0=in0, scalar=sc, in1=in1,
                    op0=mybir.AluOpType.mult, op1=mybir.AluOpType.add)

            def stt_split(out, in0, sc, in1, mtag="mm1", add_eng=None):
                """out = in0*sc + in1, as Act mul + tensor_tensor add.

                TensorScalarPtr runs at 1x on DVE (~2.4us for a 2048-elem
                f16 tile) while plain tensor_tensor runs at 2x (~1.35us);
                doing the scalar mul on the otherwise-idle Act engine nets
                ~1us of DVE per op. add_eng=nc.gpsimd pushes the add to the
                Pool Q7 cores instead."""
                m = sb.tile([128, KH4, Z], F16, tag=mtag, name=_nm("m"))
                n = in0.free_size()
                mf = m[:].rearrange("p h z -> p (h z)")[:, 0:n]
                nc.scalar.mul(out=mf, in_=in0, mul=sc)
                (add_eng or nc.vector).tensor_tensor(
                    out=out, in0=in1, in1=mf, op=mybir.AluOpType.add)

            def load_blk(field, wc, j, tag, dt_, kh=KH, eng=None):
                t_ = sb.tile([128, kh, Z], dt_, tag=tag, name=_nm("blk"))
                h0 = j * kh
                w0 = wc * 128
                # block loads dispatch from the Act queue (it has slack) so
                # the SP queue only carries the window loads
                (eng or nc.scalar).dma_start(
                    out=t_[:], in_=field[w0:w0 + 128, h0:h0 + kh, :])
                return t_

            def store_blk(field, src, wc, j, kh=KH):
                # stores ride the SP HWDGE ring (cheap dispatch); the Pool
                # Q7 cores are kept free for partial-dot compute
                h0 = j * kh
                w0 = wc * 128
                nc.sync.dma_start(
                    out=field[w0:w0 + 128, h0:h0 + kh, :], in_=src)

            def gather_v_ghosts():
                """Fill v_sb ghost rows (0 and HC+1) from haloB_out."""
                for wc in range(2):
                    nc.gpsimd.indirect_dma_start(
                        out=v_sb[wc][:, 0, :], out_offset=None,
                        in_=haloB_out[:, :],
                        in_offset=bass.IndirectOffsetOnAxis(
                            ap=idxB_sb[wc][:, 0:1], axis=0))
                    nc.gpsimd.indirect_dma_start(
                        out=v_sb[wc][:, HC + 1, :], out_offset=None,
                        in_=haloB_out[:, :],
                        in_offset=bass.IndirectOffsetOnAxis(
                            ap=idxB_sb[wc][:, 1:2], axis=0))

            def gather_r_ghosts():
                """Fill r_sb ghost rows (0 and HC+1) from haloA_out."""
                for wc in range(2):
                    nc.gpsimd.indirect_dma_start(
                        out=r_sb[wc][:, 0, :], out_offset=None,
                        in_=haloA_out[:, :],
                        in_offset=bass.IndirectOffsetOnAxis(
                            ap=idxA_sb[wc][:, 2:3], axis=0))
                    nc.gpsimd.indirect_dma_start(
                        out=r_sb[wc][:, HC + 1, :], out_offset=None,
                        in_=haloA_out[:, :],
                        in_offset=bass.IndirectOffsetOnAxis(
                            ap=idxA_sb[wc][:, 3:4], axis=0))

            if twin:
                # init big inputs so the timing loop sees normal-range fp16
                # data (uninitialized DRAM decodes to NaN/denormals)
                zi = sb.tile([128, KH, Z], F16, tag="lx", name="zinit")
                nc.vector.memset(zi[:], 0.0)
                for wc in range(2):
                    for j in range(NB):
                        store_blk(x_in, zi[:], wc, j)
                        store_blk(b_in, zi[:], wc, j)

            border = border_order(NB)
            border0 = border_order(NB0)
            # halo-producing passes: edge blocks first
            ew_order = ([0, NB - 1] if NB > 1 else [0]) + list(range(1, NB - 1))

            # ================= P0: r0 = b - S(x); rho = <r0,r0> ===========
            from contextlib import ExitStack as _ES
            _loop = _ES()
            if twin:
                _loop.enter_context(tc.For_i(0, twin_reps, 1))

            # stage x boundary planes -> haloB (free until P1 uses it)
            for wc in range(2):
                for side, h in ((0, 0), (1, HC - 1)):
                    g = sb.tile([128, Z], F16, tag="gh", name=_nm("gh"))
                    nc.sync.dma_start(
                        out=g[:], in_=x_in[wc * 128:wc * 128 + 128, h, :])
                    stage_plane(g[:], haloB_in, 0, side, wc)
            allgather(haloB_in, haloB_out, ZR_B)

            acc = None
            rho_ap = None
            if maxph >= 2:
                for j in border0:
                    wins = (load_window(x_in, 0, j, "w0a", KH0, haloB_out,
                                        idxB_sb, (0, 1), F16),
                            load_window(x_in, 1, j, "w1a", KH0, haloB_out,
                                        idxB_sb, (0, 1), F16))
                    for wc in range(2):
                        vt, vo = stencil_tile(wins, wc, j, KH0)
                        bt = load_blk(b_in, wc, j, "lx", F16, kh=KH0)
                        h0 = j * KH0
                        # r0 goes straight into the resident r field (it0
                        # has r = r0); the DRAM r0 copy feeds the dots and
                        # it0's p reads
                        r0sl = r_sb[wc][:, 1 + h0:1 + h0 + KH0, :]
                        stt_split(r0sl, vt[:, vo:vo + KH0, :], -ILAM,
                                  bt[:], "mm1")
                        acc = ttr(r0sl, r0sl, acc)
                        store_blk(fld["r0"], r0sl, wc, j, kh=KH0)
                        if j == 0:
                            stage_plane(r_sb[wc][:, 1, :], haloA_in, 0, 0, wc)
                            stage_plane(r_sb[wc][:, 1, :], haloA_in, 1, 0, wc)
                        if j == NB0 - 1:
                            stage_plane(r_sb[wc][:, HC, :], haloA_in,
                                        0, 1, wc)
                            stage_plane(r_sb[wc][:, HC, :], haloA_in,
                                        1, 1, wc)
                finish_dot(acc, 0)
                dsb = allreduce()
                rho_ap = s_scale(dsb[0:1, 0:1], ILAM)[:]
                allgather(haloA_in, haloA_out, ZR_A)
                gather_r_ghosts()

            for it in range(ITERS if maxph >= 3 else 0):
                last = (it == ITERS - 1)
                x_src = x_in if it == 0 else xw
                x_dst = x_out if last else xw
                p_src = fld["r0"] if it == 0 else fld["p"]

                # ===== P1: v = lam*S(p) -> v_sb; d1 = <r0, v> =====
                acc = None
                for j in border:
                    wins = (load_window(p_src, 0, j, "w0a", KH, haloA_out,
                                        idxA_sb, (0, 1), F16),
                            load_window(p_src, 1, j, "w1a", KH, haloA_out,
                                        idxA_sb, (0, 1), F16))
                    h0 = j * KH
                    for wc in range(2):
                        stencil_tile(wins, wc, j, KH,
                                     out_t=v_sb[wc], oh0=1 + h0)
                        lr0 = load_blk(fld["r0"], wc, j, "lr0", F16)
                        acc = ttr(lr0[:],
                                  v_sb[wc][:, 1 + h0:1 + h0 + KH, :], acc)
                        if j == 0:
                            stage_plane(v_sb[wc][:, 1, :], haloB_in, 0, 0, wc)
                        if j == NB - 1:
                            stage_plane(v_sb[wc][:, HC, :], haloB_in, 0, 1, wc)
                finish_dot(acc, 0)
                dsb = allreduce()
                allgather(haloB_in, haloB_out, ZR_B)
                gather_v_ghosts()
                d1s = s_scale(dsb[0:1, 0:1], ILAM * ILAM)
                d1_ap = d1s[:]
                alpha = s_mul(rho_ap, s_recip_eps(d1_ap)[:])
                alpha_bc = bcast(alpha[:])
                nalpha_bc = bcast(s_scale(alpha[:], -ILAM)[:])
                if maxph < 4:
                    break

                # ===== P23: s = r - alpha*v (windows built from the two
                #            resident fields, no DMA); t = lam*S(s);
                #            <t,s>, <t,t>, <r0,t> =====
                accA = accB = accC = None
                for j in border:
                    h0 = j * KH
                    swins = []
                    for wc in range(2):
                        # window rows i <-> h-plane h0-1+i <-> resident row
                        # h0+i (ghost rows included automatically at edges)
                        sw = sb.tile([128, KH + 2, Z], F16, tag=f"w{wc}a",
                                     name=_nm("sw"))
                        nc.scalar.mul(out=sw[:],
                                      in_=v_sb[wc][:, h0:h0 + KH + 2, :],
                                      mul=nalpha_bc[:])
                        nc.vector.tensor_tensor(
                            out=sw[:], in0=sw[:],
                            in1=r_sb[wc][:, h0:h0 + KH + 2, :],
                            op=mybir.AluOpType.add)
                        swins.append(sw)
                    for wc in range(2):
                        tt, to = stencil_tile(tuple(swins), wc, j, KH)
                        s_ctr = swins[wc][:, 1:KH + 1, :]
                        accA = ttr(tt[:], s_ctr, accA, "accA")
                        accB = ttr(tt[:], tt[:], accB, "accB")
                        if not last:
                            store_blk(fld["t"], tt[:], wc, j)
                            lr0 = load_blk(fld["r0"], wc, j, "lr0", F16)
                            accC = ttr(lr0[:], tt[:], accC, "accC")
                finish_dot(accA, 0)
                finish_dot(accB, 1)
                if not last:
                    finish_dot(accC, 2)
                dsb = allreduce()
                ts_s = s_scale(dsb[0:1, 0:1], ILAM * ILAM)
                tt_s = s_scale(dsb[0:1, 1:2], ILAM * ILAM * ILAM)
                omega = s_mul(ts_s[:], s_recip_eps(tt_s[:])[:])
                omega_bc = bcast(omega[:])
                nomega_bc = bcast(s_scale(omega[:], -ILAM)[:])
                if not last:
                    # rho' = (rho - alpha*d1) - omega*<r0,t>
                    r0t_s = s_scale(dsb[0:1, 2:3], ILAM * ILAM)
                    rho_n = s_sub(s_sub(rho_ap, s_mul(alpha[:], d1_ap)[:])[:],
                                  s_mul(omega[:], r0t_s[:])[:])
                    beta = s_mul(
                        s_mul(rho_n[:], s_recip_eps(rho_ap)[:])[:],
                        s_mul(alpha[:], s_recip_eps(omega[:])[:])[:])
                    beta_bc = bcast(beta[:])
                    rho_ap = rho_n[:]
                if maxph < 5:
                    break

                # ===== P45: s = r - alpha*v (from the two resident fields);
                #       x += alpha*p + omega*s;  r <- s - omega*t in place;
                #       p = r + beta*(p - omega*v) =====
                NB4 = HC // KH4
                ew4 = ([0, NB4 - 1] if NB4 > 1 else [0]) + \
                    list(range(1, NB4 - 1))
                if not last:
                    for j in ew4:
                        h0 = j * KH4
                        for wc in range(2):
                            vsl = v_sb[wc][:, 1 + h0:1 + h0 + KH4, :]
                            rsl = r_sb[wc][:, 1 + h0:1 + h0 + KH4, :]
                            tt_ = load_blk(fld["t"], wc, j, "lt", F16,
                                           kh=KH4)
                            s_t = sb.tile([128, KH4, Z], F16, tag="rt",
                                          name=_nm("st"))
                            stt_split(s_t[:], vsl, nalpha_bc[:], rsl)
                            xt = load_blk(x_src, wc, j, "lx", F16, kh=KH4)
                            pt_ = load_blk(p_src, wc, j, "lp", F16, kh=KH4)
                            # x1 = x + alpha*p, in place over the x tile
                            stt_split(xt[:], pt_[:], alpha_bc[:], xt[:])
                            x2 = sb.tile([128, KH4, Z], F16, tag="lx",
                                         name=_nm("x2"))
                            stt_split(x2[:], s_t[:], omega_bc[:], xt[:],
                                      add_eng=nc.gpsimd)
                            store_blk(x_dst, x2[:], wc, j, kh=KH4)
                            # r_new in place into the resident r field
                            stt_split(rsl, tt_[:], nomega_bc[:], s_t[:])
                            # u = p - omega*v, in place over the p tile
                            stt_split(pt_[:], vsl, nomega_bc[:], pt_[:])
                            po = sb.tile([128, KH4, Z], F16, tag="lp",
                                         name=_nm("po"))
                            stt_split(po[:], pt_[:], beta_bc[:], rsl)
                            store_blk(fld["p"], po[:], wc, j, kh=KH4)
                            if j == 0:
                                stage_plane(po[:, 0, :], haloA_in, 0, 0, wc)
                                stage_plane(r_sb[wc][:, 1, :], haloA_in,
                                            1, 0, wc)
                            if j == NB4 - 1:
                                stage_plane(po[:, KH4 - 1, :], haloA_in,
                                            0, 1, wc)
                                stage_plane(r_sb[wc][:, HC, :], haloA_in,
                                            1, 1, wc)
                    allgather(haloA_in, haloA_out, ZR_A)
                    gather_r_ghosts()
                else:
                    # last iteration: only x_out = x + alpha*p + omega*s
                    for j in range(NB4):
                        h0 = j * KH4
                        for wc in range(2):
                            vsl = v_sb[wc][:, 1 + h0:1 + h0 + KH4, :]
                            rsl = r_sb[wc][:, 1 + h0:1 + h0 + KH4, :]
                            sb_t = sb.tile([128, KH4, Z], F16, tag="rt",
                                           name=_nm("sb"))
                            stt_split(sb_t[:], vsl, nalpha_bc[:], rsl)
                            xt = load_blk(x_src, wc, j, "lx", F16, kh=KH4)
                            pt_ = load_blk(p_src, wc, j, "lp", F16, kh=KH4)
                            stt_split(xt[:], pt_[:], alpha_bc[:], xt[:])
                            x2 = sb.tile([128, KH4, Z], F32, tag="lx",
                                         name=_nm("x2"))
                            stt(x2[:], sb_t[:], omega_bc[:], xt[:])
                            store_blk(x_dst, x2[:], wc, j, kh=KH4)

                if maxph < 6 and it == 0:
                    break

            _loop.close()
            if twin:
                nc.sync.dma_start(out=dummy_out[:, :], in_=z8[:])

    nc.compile()
    return nc


# ---------------------------------------------------------------------------
# host-side wrapper
# ---------------------------------------------------------------------------
_CACHE = {}


def _shift_mats(c_w=None):
    """Stationary matrices [128, 640], scaled by LAM (exact in fp16).

    c_w given (center varies only along W): fold diag(lam*c) into the
    own-chunk matrices -> [M0 | M1 | B01n | B10n | In], PSUM = lam*S(u).
    c_w None (generic): [A | B01 | B10 | I | 0], combine on DVE.
    """
    lam = np.float32(1.0 / 256.0)
    if c_w is None:
        A = np.zeros((128, 128), np.float32)
        for i in range(127):
            A[i, i + 1] = lam
            A[i + 1, i] = lam
        B01 = np.zeros((128, 128), np.float32)
        B01[0, 127] = lam
        B10 = np.zeros((128, 128), np.float32)
        B10[127, 0] = lam
        I = lam * np.eye(128, dtype=np.float32)
        Z0 = np.zeros((128, 128), np.float32)
        return np.concatenate([A, B01, B10, I, Z0], axis=1)
    Ms = []
    for wc in range(2):
        M = np.zeros((128, 128), np.float32)
        c = np.asarray(c_w[wc * 128:(wc + 1) * 128], np.float32)
        for p in range(128):
            M[p, p] = lam * c[p]
            if p > 0:
                M[p - 1, p] = -lam
            if p < 127:
                M[p + 1, p] = -lam
        Ms.append(M)
    B01n = np.zeros((128, 128), np.float32)
    B01n[0, 127] = -lam
    B10n = np.zeros((128, 128), np.float32)
    B10n[127, 0] = -lam
    In = -lam * np.eye(128, dtype=np.float32)
    return np.concatenate([Ms[0], Ms[1], B01n, B10n, In], axis=1)


def make_const_inputs(s, HC=64, W=256, twin=False, c_w=None):
    """Per-core constant inputs (core's slab index s within its group).

    twin=True points every ghost at the zeroed rows (no collectives run, so
    halo_out buffers hold garbage that would otherwise poison fp16 timing).
    """
    matsb = _shift_mats(c_w).astype(np.float16)
    ZR_A = GROUP * 4 * W
    ZR_B = GROUP * 2 * W
    w = np.arange(W, dtype=np.int64)
    zr_a = ZR_A + (w % 128)
    zr_b = ZR_B + (w % 128)
    lo_ok = s > 0 and not twin
    hi_ok = s < GROUP - 1 and not twin
    # haloA_out: rank r rows [r*4W, (r+1)*4W); field f at f*2W; side at W
    p_lo = (s - 1) * 4 * W + 0 * 2 * W + W + w if lo_ok else zr_a
    p_hi = (s + 1) * 4 * W + 0 * 2 * W + w if hi_ok else zr_a
    r_lo = (s - 1) * 4 * W + 1 * 2 * W + W + w if lo_ok else zr_a
    r_hi = (s + 1) * 4 * W + 1 * 2 * W + w if hi_ok else zr_a
    idxA = np.stack([p_lo, p_hi, r_lo, r_hi], axis=1).astype(np.int32)
    v_lo = (s - 1) * 2 * W + W + w if lo_ok else zr_b
    v_hi = (s + 1) * 2 * W + w if hi_ok else zr_b
    idxB = np.stack([v_lo, v_hi], axis=1).astype(np.int32)
    return {"matsb": matsb, "idxA": idxA, "idxB": idxB}


def make_in_maps(x, b, center, HC, W, Z, fold):
    """Slice full inputs into per-core input maps ([W, HC, Z] layout)."""
    c_w = center[0, 0, :, 0].astype(np.float32) if fold else None
    in_maps = []
    for c in range(N_CORES):
        bi, s = divmod(c, GROUP)
        h0 = s * HC
        cen = (center[0, h0:h0 + HC, :, 0].astype(np.float32).T
               / np.float32(256.0)).copy()  # [W,HC], scaled by LAM
        m = make_const_inputs(s, HC, W, c_w=c_w)
        m.update({
            "x": np.ascontiguousarray(
                x[bi, h0:h0 + HC].transpose(1, 0, 2)).astype(np.float16),
            "bb": np.ascontiguousarray(
                b[bi, h0:h0 + HC].transpose(1, 0, 2)).astype(np.float16),
            "cen": cen,
        })
        in_maps.append(m)
    return in_maps


RUN_WALL_S = []  # wall-clock of each device dispatch (incl. axon h2d/d2h)
LAST_RESULT = None  # BassKernelResults of the most recent dispatch


def kernel(x, b, ref, center):
    """Full inputs in, full output out. ref is unused by the reference model."""
    import time as _time
    global LAST_RESULT
    B, H, W, Z = x.shape
    HC = H // GROUP
    center = np.asarray(center)
    # center varying only along W lets the diag fold into the PE stationaries
    fold = bool(np.all(center[0, :1, :, :] == center[0]))
    key = (HC, W, Z, fold)
    if key not in _CACHE:
        _CACHE[key] = build_program(HC=HC, W=W, Z=Z, fold_center=fold)
    nc = _CACHE[key]

    from concourse.bass_utils import run_bass_kernel_spmd
    in_maps = make_in_maps(np.asarray(x), np.asarray(b), center,
                           HC, W, Z, fold)
    _t0 = _time.time()
    res = run_bass_kernel_spmd(nc, in_maps, core_ids=list(range(N_CORES)))
    RUN_WALL_S.append(_time.time() - _t0)
    LAST_RESULT = res
    out = np.empty((B, H, W, Z), np.float32)
    for c in range(N_CORES):
        bi, s = divmod(c, GROUP)
        out[bi, s * HC:(s + 1) * HC] = res.results[c]["xout"].transpose(1, 0, 2)
    return out
